# revision 24
# baseline (speedup 1.0000x reference)
"""Trainium2 Bass kernel for the CPCA auxiliary loss (nn_CPCA_51754355917033).

Strategy (data-parallel over the env/batch dim n, 16 envs per core):
  - Host side: every gather is baked into per-core contiguous device
    inputs (embedding lookup folded into GIE = emb @ W_ih.T + b_ih with
    the r/z parts of b_hh also folded in, h0 gather, target gather, the
    negative gather, the forward mask and the loss denominator).
  - Device side GRU (gate dim on partitions): gi is injected into the
    gate PSUM with an identity matmul so the r/z eviction is a single
    Sigmoid straight out of PSUM; h carried in fp8; elementwise tail
    split across Vector/Scalar/GpSimd; step-u states written directly
    into the preds buffer.
  - MLP blocks: preds @ W1a + b1 computed once, re-injected per block
    into the L1 PSUM with identity matmuls, so evictions are single
    relu-cast ops.  L3 is a w3-stationary matmul giving [1, 1024]
    logits per block, copied into a flat row; at the end one reshape
    DMA spreads the row over 128 partitions for a whole-tensor
    softplus + masked-sum (one act-table switch total).
  - Host combines the 8 cores' weighted sums with the host-side
    denominator into the final scalar.
"""

import numpy as np
import ml_dtypes

import concourse.bass as bass
import concourse.mybir as mybir
import concourse.tile as tile
from concourse import bacc
from concourse import bass_utils

BF16 = ml_dtypes.bfloat16
F8 = ml_dtypes.float8_e4m3
DT = mybir.dt
AF = mybir.ActivationFunctionType
ALU = mybir.AluOpType

N, T, H, K, S, F, EMB, NLOG, NEG = 128, 512, 512, 16, 16, 4, 32, 18, 20
COEFF = 0.1
NC = 8
NPC = N // NC          # envs per core
R = NPC * S            # GRU rows per core (256)
L = T - 1
NBLK = NEG + 1         # 20 negative g-blocks + 1 positive block
BR = F * R             # rows per block (1024)
FL = NBLK * BR         # flat logits row (21504)
LB = 6                 # light blocks: L1-x matmuls interleaved into the GRU

_PROGRAM_CACHE = {}


# ----------------------------------------------------------------- host prep

def _prep_core(c, inputs, u_list, k_eff):
    acts = np.asarray(inputs["actions"])[..., 0]
    nd = np.asarray(inputs["not_dones"])[..., 0]
    ri = np.asarray(inputs["rnn_inputs"], np.float32)
    ro = np.asarray(inputs["rnn_outputs"], np.float32)
    ti = np.asarray(inputs["time_subsample"]).astype(np.int64)
    neg_idx = np.asarray(inputs["neg_idx"]).astype(np.int64)
    emb_tab = np.asarray(inputs["action_embed"], np.float32)

    ns = slice(c * NPC, (c + 1) * NPC)
    idx = np.arange(k_eff)[:, None] + ti[None, :]          # (k_eff, S)

    # gi = emb @ W_ih.T folded on host: a gather from the 18-row table.
    # GIE = action_embed @ W_ih.T + b_ih (+ b_hh for the r/z gates,
    # legal because sigmoid(ir + hr + b_ih + b_hh) is a plain sum there).
    W_ih = np.asarray(inputs["W_ih"], np.float32)
    b_ih = np.asarray(inputs["b_ih"], np.float32)
    b_hh = np.asarray(inputs["b_hh"], np.float32)
    GIE = np.zeros((NLOG + 1, 1536), np.float32)
    GIE[:NLOG] = emb_tab @ W_ih.T + b_ih
    GIE[NLOG] = b_ih
    GIE[:, :1024] += b_hh[None, :1024]
    act_ext = np.full((NPC, L + K), NLOG, np.int64)
    act_ext[:, :L] = acts[ns, :L]
    AI = act_ext[:, idx]                                   # (NPC, k_eff, S)
    gi_all = GIE[AI.transpose(1, 0, 2).reshape(k_eff, R)]  # (k_eff, R, 1536)
    giT = np.ascontiguousarray(
        gi_all.transpose(0, 2, 1).reshape(k_eff, 12, 128, R)
        .transpose(0, 2, 1, 3)).astype(BF16)               # (k_eff,128,12,R)

    H0 = ro[ns][:, ti]                                     # (NPC, S, H)
    h0T = np.ascontiguousarray(
        H0.transpose(2, 0, 1).reshape(4, 128, R)).astype(F8)

    ri_ext = np.zeros((NPC, L + K, H), np.float32)
    ri_ext[:, :L] = ri[ns, 1:]
    idx2 = np.asarray(u_list)[:, None] + ti[None, :]       # (F, S)
    TG = ri_ext[:, idx2]                                   # (NPC, F, S, H)
    tgT = np.ascontiguousarray(
        TG.transpose(3, 1, 0, 2).reshape(H, BR).reshape(4, 128, BR)).astype(F8)

    ni = neg_idx.reshape(F, N, S, NEG)[:, ns]              # (F, NPC, S, NEG)
    P = ni.transpose(3, 0, 1, 2).reshape(-1)               # cols in (g, f, j) order
    negs = ri.reshape(N * T, H)[P]
    negsT = np.ascontiguousarray(negs.T.reshape(4, 128, NEG * BR)).astype(F8)

    # forward mask + denominator on host (mask depends only on inputs)
    nd_ext = np.zeros((NPC, L + K), np.float32)
    nd_ext[:, :L] = nd[ns, :L]
    G = nd_ext[:, idx]                                     # (NPC, k_eff, S)
    ndv = G.transpose(1, 0, 2).reshape(k_eff, R) > 0       # (k_eff, R)
    cum = np.cumprod(ndv.astype(np.float32), axis=0)       # (k_eff, R)
    fm = cum[np.asarray(u_list)]                           # (F, R)
    fm_flat = fm.reshape(BR).astype(np.float32)            # f-major rows
    denom = float(fm_flat.sum())

    # weighted mask + sign in the [128, 168] transposed-flat layout:
    # tr[p, h*84 + cc] = flat[h*10752 + p*84 + cc]
    q = np.arange(FL)
    wq = fm_flat[q % BR] * np.where(q // BR < NEG, 1.0 / NEG, 1.0)
    sq = np.where(q // BR < NEG, 1.0, -1.0)
    mask168 = np.ascontiguousarray(
        wq.reshape(2, 128, 84).transpose(1, 0, 2).reshape(128, 168)
    ).astype(np.float32)
    sgn168 = np.ascontiguousarray(
        sq.reshape(2, 128, 84).transpose(1, 0, 2).reshape(128, 168)
    ).astype(np.float32)

    return dict(giT=giT, h0T8=h0T, tgT=tgT, negsT=negsT,
                mask168=mask168, sgn168=sgn168), denom


def _prep_weights(inputs):
    W_hh = np.asarray(inputs["W_hh"], np.float32)
    b_hh = np.asarray(inputs["b_hh"], np.float32)
    W1 = np.asarray(inputs["W1"], np.float32)
    b1 = np.asarray(inputs["b1"], np.float32)
    W2 = np.asarray(inputs["W2"], np.float32)
    b2 = np.asarray(inputs["b2"], np.float32)
    W3 = np.asarray(inputs["W3"], np.float32)
    b3 = np.asarray(inputs["b3"], np.float32)

    d = {}
    d["w_hh8"] = np.ascontiguousarray(
        W_hh.T.reshape(2, 2, 128, 1536).transpose(0, 2, 1, 3)).astype(F8)
    def pack8(WT):
        # [t, ki, ko, m] with contract index = t*256 + ko*128 + ki
        return np.ascontiguousarray(
            WT.reshape(2, 2, 128, WT.shape[1]).transpose(0, 2, 1, 3)).astype(F8)
    d["w1a8"] = pack8(W1[:, :512].T.copy())
    d["w1b8"] = pack8(W1[:, 512:].T.copy())
    d["w28"] = pack8(W2.T.copy())
    d["w3T"] = np.ascontiguousarray(W3[0].reshape(4, 128).T).astype(BF16)
    # DoubleRow-packed w3: [t, ki, ko, 16] (cols 1..15 zero-padded so the
    # dual-fp8 ldweights stride restrictions hold); contract = t*256+ko*128+ki
    w3dr = np.zeros((2, 128, 2, 16), np.float32)
    w3dr[:, :, :, 0] = W3[0].reshape(2, 2, 128).transpose(0, 2, 1)
    d["w3dr"] = w3dr.astype(F8)
    # n-gate hidden biases only (r/z parts folded into GIE on host)
    bgn = np.zeros((128, 4), np.float32)
    for cc in range(4):
        bgn[:, cc] = b_hh[1024 + cc * 128:1024 + (cc + 1) * 128]
    d["bgn"] = bgn
    d["b1T"] = np.ascontiguousarray(b1.reshape(4, 128).T).astype(np.float32)
    d["b2T"] = np.ascontiguousarray(b2.reshape(4, 128).T).astype(np.float32)
    d["idbf"] = np.eye(128, dtype=BF16)
    d["id8"] = np.eye(128, dtype=F8)
    d["b3f"] = float(b3.reshape(-1)[0])
    return d


# ------------------------------------------------------------- device program

def _build_program(u_list, k_eff, b3f, upto=4):
    nc = bacc.Bacc("TRN2", target_bir_lowering=False, debug=False, num_devices=NC)

    di = {}
    def inp(name, shape, dt):
        di[name] = nc.dram_tensor(name, list(shape), dt, kind="ExternalInput")
        return di[name]

    d_whh = inp("w_hh8", (2, 128, 2, 1536), DT.float8e4)
    d_w1a = inp("w1a8", (2, 128, 2, 512), DT.float8e4)
    d_w1b = inp("w1b8", (2, 128, 2, 512), DT.float8e4)
    d_w2 = inp("w28", (2, 128, 2, 512), DT.float8e4)
    d_w3 = inp("w3T", (128, 4), DT.bfloat16)
    d_w3dr = inp("w3dr", (2, 128, 2, 16), DT.float8e4)
    d_bgn = inp("bgn", (128, 4), DT.float32)
    d_b1 = inp("b1T", (128, 4), DT.float32)
    d_b2 = inp("b2T", (128, 4), DT.float32)
    d_idbf = inp("idbf", (128, 128), DT.bfloat16)
    d_id8 = inp("id8", (128, 128), DT.float8e4)
    d_gi = inp("giT", (k_eff, 128, 12, R), DT.bfloat16)
    d_h0 = inp("h0T8", (4, 128, R), DT.float8e4)
    d_tg = inp("tgT", (4, 128, BR), DT.float8e4)
    d_negs = inp("negsT", (4, 128, NEG * BR), DT.float8e4)
    d_m168 = inp("mask168", (128, 168), DT.float32)
    d_s168 = inp("sgn168", (128, 168), DT.float32)
    d_out = nc.dram_tensor("out", [1, 4], DT.float32, kind="ExternalOutput")

    f32 = DT.float32
    bf16 = DT.bfloat16
    f8 = DT.float8e4
    DR = mybir.MatmulPerfMode.DoubleRow

    # which GRU step each light block's (b, cc) L1-x group is issued after
    inter_sched = {}
    if k_eff >= 2:
        pairs = [(b, cc) for b in range(LB) for cc in range(4)]
        steps = list(range(1, k_eff))
        for i, (b, cc) in enumerate(pairs):
            st = steps[min(int(i * len(steps) / len(pairs)), len(steps) - 1)]
            inter_sched.setdefault(st, []).append((b, cc))

    with tile.TileContext(nc) as tc:
        with (
            tc.tile_pool(name="const", bufs=1) as cp,
            tc.tile_pool(name="gruw", bufs=2) as gp,
            tc.tile_pool(name="mlpw", bufs=2) as mp,
            tc.tile_pool(name="psg", bufs=2, space="PSUM") as pg,
            tc.tile_pool(name="psm", bufs=2, space="PSUM") as pm,
        ):
            # ------------------------------------------------ constant loads
            # single transposed-AP DMAs; only GRU-critical ones up front,
            # the rest deferred into the first two GRU steps (the Sync
            # engine issues dma_starts serially at ~0.65us each)
            idbf = cp.tile([128, 128], bf16, tag="idbf")
            nc.sync.dma_start(out=idbf[:], in_=d_idbf[:])
            h0 = cp.tile([128, 4, R], f8, tag="h0")
            nc.sync.dma_start(out=h0[:], in_=d_h0[:].transpose([1, 0, 2]))
            whh = cp.tile([128, 2, 2, 1536], f8, tag="whh")
            nc.sync.dma_start(out=whh[:], in_=d_whh[:].transpose([1, 0, 2, 3]))
            bgn = cp.tile([128, 4], f32, tag="bgn")
            nc.sync.dma_start(out=bgn[:], in_=d_bgn[:])
            id8 = cp.tile([128, 128], f8, tag="id8")
            b1 = cp.tile([128, 4], f32, tag="b1")
            b2 = cp.tile([128, 4], f32, tag="b2")
            w3 = cp.tile([128, 4], bf16, tag="w3")
            w3dr = cp.tile([128, 2, 2, 16], f8, tag="w3dr")
            m168 = cp.tile([128, 168], f32, tag="m168")
            s168 = cp.tile([128, 168], f32, tag="s168")
            w1a = cp.tile([128, 2, 2, 512], f8, tag="w1a")
            w1b = cp.tile([128, 2, 2, 512], f8, tag="w1b")
            w2 = cp.tile([128, 2, 2, 512], f8, tag="w2")
            tg = cp.tile([128, 4, BR], f8, tag="tg")

            def deferred_loads(stage):
                if stage == 0:
                    nc.sync.dma_start(out=w1b[:],
                                      in_=d_w1b[:].transpose([1, 0, 2, 3]))
                    nc.sync.dma_start(out=id8[:], in_=d_id8[:])
                    nc.sync.dma_start(out=b1[:], in_=d_b1[:])
                    nc.sync.dma_start(out=b2[:], in_=d_b2[:])
                elif stage == 1:
                    nc.sync.dma_start(out=w2[:],
                                      in_=d_w2[:].transpose([1, 0, 2, 3]))
                    nc.sync.dma_start(out=w1a[:],
                                      in_=d_w1a[:].transpose([1, 0, 2, 3]))
                    nc.sync.dma_start(out=w3[:], in_=d_w3[:])
                    nc.sync.dma_start(out=w3dr[:],
                                      in_=d_w3dr[:].transpose([1, 0, 2, 3]))
                elif stage == 2:
                    nc.sync.dma_start(out=tg[:],
                                      in_=d_tg[:].transpose([1, 0, 2]))
                    nc.sync.dma_start(out=m168[:], in_=d_m168[:])
                    nc.sync.dma_start(out=s168[:], in_=d_s168[:])

            # parked L1-x pre-activations for the light blocks; their fp8
            # inputs stream through the same pool tag the full blocks use
            xb1 = [cp.tile([128, 4, 2, 512], f8, tag=f"xb1_{b}", name=f"xb1_{b}")
                   for b in range(LB)]
            xlb_cur = {}

            def light_xt(b):
                if b not in xlb_cur:
                    xt = mp.tile([128, 4, BR], f8, tag="negsx", bufs=2,
                                 name=f"negsx_lb{b}")
                    nc.sync.dma_start(
                        out=xt[:],
                        in_=d_negs[:, :, b * BR:(b + 1) * BR].transpose([1, 0, 2]))
                    xlb_cur[b] = xt
                return xlb_cur[b]

            predsT = cp.tile([128, 4, BR], f8, tag="preds")
            onesb = cp.tile([128, 4, R], bf16, tag="onesb")
            nc.gpsimd.memset(onesb[:], 1.0)

            # ------------------------------------------------ GRU
            # th-split: all th=0 matmuls of a step (which need only the
            # early half of h) are issued before any th=1 matmul, so the
            # previous step's elementwise tail overlaps real matmul work.
            do_gru = upto >= 1
            h_prev = h0
            for k in range(k_eff if do_gru else 0):
                gi = gp.tile([128, 12, R], bf16, tag="gi", bufs=2)
                nc.sync.dma_start(out=gi[:], in_=d_gi[k])

                rza = gp.tile([128, 8, R], bf16, tag="rza", bufs=1)
                rzps = [pg.tile([128, 4, R], f32, tag="pg", name=f"rzps{q}_{k}")
                        for q in range(2)]
                # th=0 wave (needs h half 0 only) + the gi identity-inject
                for q in range(2):
                    for j in range(4):
                        gc = 4 * q + j
                        nc.tensor.matmul(rzps[q][:, j, :], idbf[:],
                                         gi[:, gc, :], start=True, stop=False)
                        nc.tensor.matmul(
                            rzps[q][:, j, :], whh[:, 0, :, gc * 128:(gc + 1) * 128],
                            h_prev[:, 0:2, :],
                            start=False, stop=False, perf_mode=DR)
                # th=1 wave + sigmoid eviction per quad
                for q in range(2):
                    for j in range(4):
                        gc = 4 * q + j
                        nc.tensor.matmul(
                            rzps[q][:, j, :], whh[:, 1, :, gc * 128:(gc + 1) * 128],
                            h_prev[:, 2:4, :],
                            start=False, stop=True, perf_mode=DR)
                    nc.scalar.activation(rza[:, 4 * q:4 * q + 4, :], rzps[q][:],
                                         AF.Sigmoid)
                r_sb = rza[:, 0:4, :]
                z_sb = rza[:, 4:8, :]
                e_sb = gp.tile([128, 4, R], bf16, tag="e", bufs=1)
                nc.vector.tensor_mul(e_sb[:, 0:2, :], rza[:, 4:6, :],
                                     h_prev[:, 0:2, :])
                w1m = gp.tile([128, 4, R], bf16, tag="w1m", bufs=1)
                nc.scalar.activation(w1m[:], z_sb, AF.Identity,
                                     scale=-1.0, bias=1.0)

                # n gates: one quad group, th-split as well
                nps = pg.tile([128, 4, R], f32, tag="pg", name=f"nps_{k}")
                for j in range(4):
                    nc.tensor.matmul(
                        nps[:, j, :], whh[:, 0, :, (8 + j) * 128:(9 + j) * 128],
                        h_prev[:, 0:2, :], start=True, stop=False, perf_mode=DR)
                for j in range(4):
                    nc.tensor.matmul(
                        nps[:, j, :], whh[:, 1, :, (8 + j) * 128:(9 + j) * 128],
                        h_prev[:, 2:4, :], start=False, stop=True, perf_mode=DR)

                t_sb = gp.tile([128, 4, R], f32, tag="t", bufs=1)
                u_sb = gp.tile([128, 4, R], bf16, tag="u", bufs=1)
                g_sb = gp.tile([128, 4, R], bf16, tag="g", bufs=1)
                gw = gp.tile([128, 4, R], bf16, tag="gw", bufs=1)

                us = [fi for fi, u in enumerate(u_list) if u == k]
                if us:
                    fi0 = us[0]
                    h_new = predsT[:, :, fi0 * R:(fi0 + 1) * R]
                else:
                    h_new = gp.tile([128, 4, R], f8, tag="h8")

                # half 0 chain on Vector (it gates the next step's th=0
                # wave); half 1 has the th=0 wave's slack: GpSimd
                for pi in range(2):
                    for j in range(2):
                        c = 2 * pi + j
                        nc.vector.scalar_tensor_tensor(
                            t_sb[:, c, :], in0=nps[:, c, :], scalar=bgn[:, c:c + 1],
                            in1=r_sb[:, c, :], op0=ALU.add, op1=ALU.mult)
                        if pi == 0:
                            nc.vector.tensor_add(u_sb[:, c, :], gi[:, 8 + c, :],
                                                 t_sb[:, c, :])
                        else:
                            nc.gpsimd.tensor_add(u_sb[:, c, :], gi[:, 8 + c, :],
                                                 t_sb[:, c, :])
                    hh = slice(2 * pi, 2 * pi + 2)
                    nc.scalar.activation(g_sb[:, hh, :], u_sb[:, hh, :], AF.Tanh)
                    if pi == 0:
                        nc.vector.tensor_mul(gw[:, hh, :], g_sb[:, hh, :],
                                             w1m[:, hh, :])
                    else:
                        nc.gpsimd.tensor_mul(e_sb[:, 2:4, :], rza[:, 6:8, :],
                                             h_prev[:, 2:4, :])
                        nc.gpsimd.tensor_mul(gw[:, hh, :], g_sb[:, hh, :],
                                             w1m[:, hh, :])
                    nc.vector.tensor_add(h_new[:, hh, :], gw[:, hh, :],
                                         e_sb[:, hh, :])
                for fi in us[1:]:
                    nc.vector.tensor_copy(
                        predsT[:, :, fi * R:(fi + 1) * R], h_new[:])
                h_prev = h_new

                # interleaved light-block L1-x matmuls (fill the PE bubble)
                for (b, cc) in inter_sched.get(k, []):
                    xt = light_xt(b)
                    ps = pm.tile([128, 2, 512], f32, tag="pm")
                    for rt in range(2):
                        sl = slice(rt * 512, (rt + 1) * 512)
                        for th in range(2):
                            nc.tensor.matmul(
                                ps[:, rt, :],
                                w1b[:, th, :, cc * 128:(cc + 1) * 128],
                                xt[:, 2 * th:2 * th + 2, sl],
                                start=(th == 0), stop=(th == 1), perf_mode=DR)
                    if cc % 2 == 0:
                        nc.scalar.activation(xb1[b][:, cc, :, :], ps[:], AF.Copy)
                    else:
                        nc.vector.tensor_copy(xb1[b][:, cc, :, :], ps[:])
                deferred_loads(k)

            # ------------------------------------------- preds @ W1a + b1 cache
            cach = cp.tile([128, 4, 2, 512], bf16, tag="cach")
            if upto >= 2:
                for cc in range(4):
                    ps = pm.tile([128, 2, 512], f32, tag="pm")
                    for rt in range(2):
                        sl = slice(rt * 512, (rt + 1) * 512)
                        for th in range(2):
                            nc.tensor.matmul(
                                ps[:, rt, :],
                                w1a[:, th, :, cc * 128:(cc + 1) * 128],
                                predsT[:, 2 * th:2 * th + 2, sl],
                                start=(th == 0), stop=(th == 1), perf_mode=DR)
                    nc.scalar.activation(cach[:, cc, :, :], ps[:], AF.Identity,
                                         bias=b1[:, cc:cc + 1])

            # ------------------------------------------------ blocks
            flat = cp.tile([1, FL], bf16, tag="flat")
            tr168 = cp.tile([128, 168], bf16, tag="tr168")

            def l1_full(b, xt):
                y1 = mp.tile([128, 4, BR], f8, tag="y1", bufs=2)
                for cc in range(4):
                    ps = pm.tile([128, 2, 512], f32, tag="pm")
                    for rt in range(2):
                        sl = slice(rt * 512, (rt + 1) * 512)
                        nc.tensor.matmul(ps[:, rt, :], idbf[:],
                                         cach[:, cc, rt, :], start=True,
                                         stop=False)
                        for th in range(2):
                            nc.tensor.matmul(
                                ps[:, rt, :],
                                w1b[:, th, :, cc * 128:(cc + 1) * 128],
                                xt[:, 2 * th:2 * th + 2, sl],
                                start=False, stop=(th == 1), perf_mode=DR)
                    nc.vector.tensor_scalar(y1[:, cc, :], ps[:], 0.0, None,
                                            op0=ALU.max)
                return y1

            def l1_light(b):
                y1 = mp.tile([128, 4, BR], f8, tag="y1", bufs=2)
                for cc in range(4):
                    ps = pm.tile([128, 2, 512], f32, tag="pm")
                    for rt in range(2):
                        nc.tensor.matmul(ps[:, rt, :], id8[:],
                                         xb1[b][:, cc, rt, :], start=True,
                                         stop=False)
                        nc.tensor.matmul(ps[:, rt, :], idbf[:],
                                         cach[:, cc, rt, :], start=False,
                                         stop=True)
                    nc.vector.tensor_scalar(y1[:, cc, :], ps[:], 0.0, None,
                                            op0=ALU.max)
                return y1

            def l2_l3(b, y1):
                y2 = mp.tile([128, 4, BR], f8, tag="y2", bufs=2)
                for cc in range(4):
                    ps = pm.tile([128, 2, 512], f32, tag="pm")
                    for rt in range(2):
                        sl = slice(rt * 512, (rt + 1) * 512)
                        for th in range(2):
                            nc.tensor.matmul(
                                ps[:, rt, :],
                                w2[:, th, :, cc * 128:(cc + 1) * 128],
                                y1[:, 2 * th:2 * th + 2, sl],
                                start=(th == 0), stop=(th == 1), perf_mode=DR)
                    if cc < 2:
                        nc.vector.tensor_scalar(y2[:, cc, :], ps[:],
                                                b2[:, cc:cc + 1], 0.0,
                                                op0=ALU.add, op1=ALU.max)
                    else:
                        nc.scalar.activation(y2[:, cc, :], ps[:], AF.Relu,
                                             bias=b2[:, cc:cc + 1])
                # L3: w3-stationary DoubleRow, [1, 512] psum per half
                ps3 = pm.tile([128, 2, 512], f32, tag="pm", name=f"ps3_{b}")
                for j in range(2):
                    sl = slice(j * 512, (j + 1) * 512)
                    for t3 in range(2):
                        nc.tensor.matmul(
                            ps3[0:16, j, :], w3dr[:, t3, :, :],
                            y2[:, 2 * t3:2 * t3 + 2, sl],
                            start=(t3 == 0), stop=(t3 == 1), perf_mode=DR)
                nc.scalar.activation(flat[0:1, b * BR:(b + 1) * BR],
                                     ps3[0:1, :, :], AF.Copy)

            if upto >= 3:
                prev = None
                for b in range(NBLK):
                    if b < LB:
                        y1 = l1_light(b)
                    else:
                        if b < NEG:
                            xt = mp.tile([128, 4, BR], f8, tag="negsx", bufs=2)
                            for kc in range(4):
                                nc.sync.dma_start(
                                    out=xt[:, kc, :],
                                    in_=d_negs[kc][:, b * BR:(b + 1) * BR])
                        else:
                            xt = tg
                        y1 = l1_full(b, xt)
                    if prev is not None:
                        l2_l3(*prev)
                        # reshape-DMA half 0 once blocks 0..10 are flat
                        if prev[0] == 10:
                            for q in range(4):
                                nc.sync.dma_start(
                                    out=tr168[q * 32:(q + 1) * 32, 0:84],
                                    in_=flat[0:1, q * 2688:(q + 1) * 2688])
                    prev = (b, y1)
                l2_l3(*prev)
                for q in range(4):
                    nc.sync.dma_start(
                        out=tr168[q * 32:(q + 1) * 32, 84:168],
                        in_=flat[0:1, 10752 + q * 2688:10752 + (q + 1) * 2688])

            # --------------------------------------- softplus + masked sum
            # softplus(s*t) = relu(s*t) - ln(sigmoid(|t|)); one table switch
            out_sb = cp.tile([1, 4], f32, tag="out_sb")
            if upto >= 4:
                st = cp.tile([128, 168], f32, tag="st")
                nc.vector.scalar_tensor_tensor(
                    st[:], in0=tr168[:], scalar=b3f, in1=s168[:],
                    op0=ALU.add, op1=ALU.mult)
                ab = cp.tile([128, 168], f32, tag="ab")
                nc.scalar.activation(ab[:], st[:], AF.Abs)
                sg = cp.tile([128, 168], f32, tag="sg")
                nc.scalar.activation(sg[:], ab[:], AF.Sigmoid)
                ln = cp.tile([128, 168], f32, tag="ln")
                nc.scalar.activation(ln[:], sg[:], AF.Ln)
                rl = cp.tile([128, 168], f32, tag="rl")
                nc.vector.tensor_scalar(rl[:], st[:], 0.0, None, op0=ALU.max)
                sp = cp.tile([128, 168], f32, tag="sp")
                nc.vector.scalar_tensor_tensor(
                    sp[:], in0=ln[:], scalar=-1.0, in1=rl[:],
                    op0=ALU.mult, op1=ALU.add)
                wsp = cp.tile([128, 168], f32, tag="wsp")
                rsum = cp.tile([128, 1], f32, tag="rsum")
                nc.vector.scalar_tensor_tensor(
                    wsp[:], in0=sp[:], scalar=1.0, in1=m168[:],
                    op0=ALU.mult, op1=ALU.mult, accum_out=rsum[:])
                ones = cp.tile([128, 1], f32, tag="ones")
                nc.vector.memset(ones[:], 1.0)
                psf = pm.tile([128, 2, 512], f32, tag="pm", name="psf")
                nc.tensor.matmul(psf[0:1, 0, 0:1], rsum[:], ones[:],
                                 start=True, stop=True)
                nc.scalar.activation(out_sb[:], psf[0:1, 0, 0:4], AF.Copy)
            else:
                nc.vector.memset(out_sb[:], 0.0)
            nc.sync.dma_start(out=d_out[:], in_=out_sb[:])

    nc.finalize()
    return nc


def _get_program(u_list, k_eff, b3f):
    key = (tuple(u_list), k_eff, float(b3f))
    if key not in _PROGRAM_CACHE:
        _PROGRAM_CACHE[key] = _build_program(u_list, k_eff, b3f)
    return _PROGRAM_CACHE[key]


# ------------------------------------------------------------------ kernel

def kernel(**inputs):
    u_list = [int(x) for x in np.asarray(inputs["unroll_subsample"]).reshape(-1)]
    k_eff = max(u_list) + 1
    w = _prep_weights(inputs)
    nc = _get_program(u_list, k_eff, w["b3f"])

    wmaps = {k: v for k, v in w.items() if k != "b3f"}
    in_maps = []
    D = 0.0
    for c in range(NC):
        m = dict(wmaps)
        cm, dc = _prep_core(c, inputs, u_list, k_eff)
        m.update(cm)
        in_maps.append(m)
        D += dc

    res = bass_utils.run_bass_kernel_spmd(nc, in_maps, list(range(NC)))
    WS = 0.0
    for c in range(NC):
        o = np.asarray(res.results[c]["out"], np.float64)
        WS += o[0, 0]
    loss = COEFF * WS / D
    return np.float32(loss)


# revision 25
# speedup vs baseline: 1.0150x; 1.0150x over previous
"""Trainium2 Bass kernel for the CPCA auxiliary loss (nn_CPCA_51754355917033).

Strategy (data-parallel over the env/batch dim n, 16 envs per core):
  - Host side: every gather is baked into per-core contiguous device
    inputs (embedding lookup folded into GIE = emb @ W_ih.T + b_ih with
    the r/z parts of b_hh also folded in, h0 gather, target gather, the
    negative gather, the forward mask and the loss denominator).
  - Device side GRU (gate dim on partitions): gi is injected into the
    gate PSUM with an identity matmul so the r/z eviction is a single
    Sigmoid straight out of PSUM; h carried in fp8; elementwise tail
    split across Vector/Scalar/GpSimd; step-u states written directly
    into the preds buffer.
  - MLP blocks: preds @ W1a + b1 computed once, re-injected per block
    into the L1 PSUM with identity matmuls, so evictions are single
    relu-cast ops.  L3 is a w3-stationary matmul giving [1, 1024]
    logits per block, copied into a flat row; at the end one reshape
    DMA spreads the row over 128 partitions for a whole-tensor
    softplus + masked-sum (one act-table switch total).
  - Host combines the 8 cores' weighted sums with the host-side
    denominator into the final scalar.
"""

import numpy as np
import ml_dtypes

import concourse.bass as bass
import concourse.mybir as mybir
import concourse.tile as tile
from concourse import bacc
from concourse import bass_utils

BF16 = ml_dtypes.bfloat16
F8 = ml_dtypes.float8_e4m3
DT = mybir.dt
AF = mybir.ActivationFunctionType
ALU = mybir.AluOpType

N, T, H, K, S, F, EMB, NLOG, NEG = 128, 512, 512, 16, 16, 4, 32, 18, 20
COEFF = 0.1
NC = 8
NPC = N // NC          # envs per core
R = NPC * S            # GRU rows per core (256)
L = T - 1
NBLK = NEG + 1         # 20 negative g-blocks + 1 positive block
BR = F * R             # rows per block (1024)
FL = NBLK * BR         # flat logits row (21504)
LB = 10                # light blocks: L1-x matmuls interleaved into the GRU

_PROGRAM_CACHE = {}


# ----------------------------------------------------------------- host prep

def _prep_core(c, inputs, u_list, k_eff):
    acts = np.asarray(inputs["actions"])[..., 0]
    nd = np.asarray(inputs["not_dones"])[..., 0]
    ri = np.asarray(inputs["rnn_inputs"], np.float32)
    ro = np.asarray(inputs["rnn_outputs"], np.float32)
    ti = np.asarray(inputs["time_subsample"]).astype(np.int64)
    neg_idx = np.asarray(inputs["neg_idx"]).astype(np.int64)
    emb_tab = np.asarray(inputs["action_embed"], np.float32)

    ns = slice(c * NPC, (c + 1) * NPC)
    idx = np.arange(k_eff)[:, None] + ti[None, :]          # (k_eff, S)

    # gi = emb @ W_ih.T folded on host: a gather from the 18-row table.
    # GIE = action_embed @ W_ih.T + b_ih (+ b_hh for the r/z gates,
    # legal because sigmoid(ir + hr + b_ih + b_hh) is a plain sum there).
    W_ih = np.asarray(inputs["W_ih"], np.float32)
    b_ih = np.asarray(inputs["b_ih"], np.float32)
    b_hh = np.asarray(inputs["b_hh"], np.float32)
    GIE = np.zeros((NLOG + 1, 1536), np.float32)
    GIE[:NLOG] = emb_tab @ W_ih.T + b_ih
    GIE[NLOG] = b_ih
    GIE[:, :1024] += b_hh[None, :1024]
    act_ext = np.full((NPC, L + K), NLOG, np.int64)
    act_ext[:, :L] = acts[ns, :L]
    AI = act_ext[:, idx]                                   # (NPC, k_eff, S)
    gi_all = GIE[AI.transpose(1, 0, 2).reshape(k_eff, R)]  # (k_eff, R, 1536)
    giT = np.ascontiguousarray(
        gi_all.transpose(0, 2, 1).reshape(k_eff, 12, 128, R)
        .transpose(0, 2, 1, 3)).astype(BF16)               # (k_eff,128,12,R)

    H0 = ro[ns][:, ti]                                     # (NPC, S, H)
    h0T = np.ascontiguousarray(
        H0.transpose(2, 0, 1).reshape(4, 128, R)).astype(F8)

    ri_ext = np.zeros((NPC, L + K, H), np.float32)
    ri_ext[:, :L] = ri[ns, 1:]
    idx2 = np.asarray(u_list)[:, None] + ti[None, :]       # (F, S)
    TG = ri_ext[:, idx2]                                   # (NPC, F, S, H)
    tgT = np.ascontiguousarray(
        TG.transpose(3, 1, 0, 2).reshape(H, BR).reshape(4, 128, BR)).astype(F8)

    ni = neg_idx.reshape(F, N, S, NEG)[:, ns]              # (F, NPC, S, NEG)
    P = ni.transpose(3, 0, 1, 2).reshape(-1)               # cols in (g, f, j) order
    negs = ri.reshape(N * T, H)[P]
    negsT = np.ascontiguousarray(negs.T.reshape(4, 128, NEG * BR)).astype(F8)

    # forward mask + denominator on host (mask depends only on inputs)
    nd_ext = np.zeros((NPC, L + K), np.float32)
    nd_ext[:, :L] = nd[ns, :L]
    G = nd_ext[:, idx]                                     # (NPC, k_eff, S)
    ndv = G.transpose(1, 0, 2).reshape(k_eff, R) > 0       # (k_eff, R)
    cum = np.cumprod(ndv.astype(np.float32), axis=0)       # (k_eff, R)
    fm = cum[np.asarray(u_list)]                           # (F, R)
    fm_flat = fm.reshape(BR).astype(np.float32)            # f-major rows
    denom = float(fm_flat.sum())

    # weighted mask + sign in the [128, 168] transposed-flat layout:
    # tr[p, h*84 + cc] = flat[h*10752 + p*84 + cc]
    q = np.arange(FL)
    wq = fm_flat[q % BR] * np.where(q // BR < NEG, 1.0 / NEG, 1.0)
    sq = np.where(q // BR < NEG, 1.0, -1.0)
    mask168 = np.ascontiguousarray(
        wq.reshape(2, 128, 84).transpose(1, 0, 2).reshape(128, 168)
    ).astype(np.float32)
    sgn168 = np.ascontiguousarray(
        sq.reshape(2, 128, 84).transpose(1, 0, 2).reshape(128, 168)
    ).astype(np.float32)

    return dict(giT=giT, h0T8=h0T, tgT=tgT, negsT=negsT,
                mask168=mask168, sgn168=sgn168), denom


def _prep_weights(inputs):
    W_hh = np.asarray(inputs["W_hh"], np.float32)
    b_hh = np.asarray(inputs["b_hh"], np.float32)
    W1 = np.asarray(inputs["W1"], np.float32)
    b1 = np.asarray(inputs["b1"], np.float32)
    W2 = np.asarray(inputs["W2"], np.float32)
    b2 = np.asarray(inputs["b2"], np.float32)
    W3 = np.asarray(inputs["W3"], np.float32)
    b3 = np.asarray(inputs["b3"], np.float32)

    d = {}
    d["w_hh8"] = np.ascontiguousarray(
        W_hh.T.reshape(2, 2, 128, 1536).transpose(0, 2, 1, 3)).astype(F8)
    def pack8(WT):
        # [t, ki, ko, m] with contract index = t*256 + ko*128 + ki
        return np.ascontiguousarray(
            WT.reshape(2, 2, 128, WT.shape[1]).transpose(0, 2, 1, 3)).astype(F8)
    d["w1a8"] = pack8(W1[:, :512].T.copy())
    d["w1b8"] = pack8(W1[:, 512:].T.copy())
    d["w28"] = pack8(W2.T.copy())
    d["w3T"] = np.ascontiguousarray(W3[0].reshape(4, 128).T).astype(BF16)
    # DoubleRow-packed w3: [t, ki, ko, 16] (cols 1..15 zero-padded so the
    # dual-fp8 ldweights stride restrictions hold); contract = t*256+ko*128+ki
    w3dr = np.zeros((2, 128, 2, 16), np.float32)
    w3dr[:, :, :, 0] = W3[0].reshape(2, 2, 128).transpose(0, 2, 1)
    d["w3dr"] = w3dr.astype(F8)
    # n-gate hidden biases only (r/z parts folded into GIE on host)
    bgn = np.zeros((128, 4), np.float32)
    for cc in range(4):
        bgn[:, cc] = b_hh[1024 + cc * 128:1024 + (cc + 1) * 128]
    d["bgn"] = bgn
    d["b1T"] = np.ascontiguousarray(b1.reshape(4, 128).T).astype(np.float32)
    d["b2T"] = np.ascontiguousarray(b2.reshape(4, 128).T).astype(np.float32)
    d["idbf"] = np.eye(128, dtype=BF16)
    d["id8"] = np.eye(128, dtype=F8)
    d["b3f"] = float(b3.reshape(-1)[0])
    return d


# ------------------------------------------------------------- device program

def _build_program(u_list, k_eff, b3f, upto=4):
    nc = bacc.Bacc("TRN2", target_bir_lowering=False, debug=False, num_devices=NC)

    di = {}
    def inp(name, shape, dt):
        di[name] = nc.dram_tensor(name, list(shape), dt, kind="ExternalInput")
        return di[name]

    d_whh = inp("w_hh8", (2, 128, 2, 1536), DT.float8e4)
    d_w1a = inp("w1a8", (2, 128, 2, 512), DT.float8e4)
    d_w1b = inp("w1b8", (2, 128, 2, 512), DT.float8e4)
    d_w2 = inp("w28", (2, 128, 2, 512), DT.float8e4)
    d_w3 = inp("w3T", (128, 4), DT.bfloat16)
    d_w3dr = inp("w3dr", (2, 128, 2, 16), DT.float8e4)
    d_bgn = inp("bgn", (128, 4), DT.float32)
    d_b1 = inp("b1T", (128, 4), DT.float32)
    d_b2 = inp("b2T", (128, 4), DT.float32)
    d_idbf = inp("idbf", (128, 128), DT.bfloat16)
    d_id8 = inp("id8", (128, 128), DT.float8e4)
    d_gi = inp("giT", (k_eff, 128, 12, R), DT.bfloat16)
    d_h0 = inp("h0T8", (4, 128, R), DT.float8e4)
    d_tg = inp("tgT", (4, 128, BR), DT.float8e4)
    d_negs = inp("negsT", (4, 128, NEG * BR), DT.float8e4)
    d_m168 = inp("mask168", (128, 168), DT.float32)
    d_s168 = inp("sgn168", (128, 168), DT.float32)
    d_out = nc.dram_tensor("out", [1, 4], DT.float32, kind="ExternalOutput")

    f32 = DT.float32
    bf16 = DT.bfloat16
    f8 = DT.float8e4
    DR = mybir.MatmulPerfMode.DoubleRow

    # which GRU step each light block's (b, cc) L1-x group is issued after
    inter_sched = {}
    if k_eff >= 2:
        pairs = [(b, cc) for b in range(LB) for cc in range(4)]
        steps = list(range(1, k_eff))
        for i, (b, cc) in enumerate(pairs):
            st = steps[min(int(i * len(steps) / len(pairs)), len(steps) - 1)]
            inter_sched.setdefault(st, []).append((b, cc))

    with tile.TileContext(nc) as tc:
        with (
            tc.tile_pool(name="const", bufs=1) as cp,
            tc.tile_pool(name="gruw", bufs=2) as gp,
            tc.tile_pool(name="mlpw", bufs=2) as mp,
            tc.tile_pool(name="psg", bufs=2, space="PSUM") as pg,
            tc.tile_pool(name="psm", bufs=2, space="PSUM") as pm,
        ):
            # ------------------------------------------------ constant loads
            # single transposed-AP DMAs; only GRU-critical ones up front,
            # the rest deferred into the first two GRU steps (the Sync
            # engine issues dma_starts serially at ~0.65us each)
            idbf = cp.tile([128, 128], bf16, tag="idbf")
            nc.sync.dma_start(out=idbf[:], in_=d_idbf[:])
            h0 = cp.tile([128, 4, R], f8, tag="h0")
            nc.sync.dma_start(out=h0[:], in_=d_h0[:].transpose([1, 0, 2]))
            whh = cp.tile([128, 2, 2, 1536], f8, tag="whh")
            nc.sync.dma_start(out=whh[:], in_=d_whh[:].transpose([1, 0, 2, 3]))
            bgn = cp.tile([128, 4], f32, tag="bgn")
            nc.sync.dma_start(out=bgn[:], in_=d_bgn[:])
            id8 = cp.tile([128, 128], f8, tag="id8")
            b1 = cp.tile([128, 4], f32, tag="b1")
            b2 = cp.tile([128, 4], f32, tag="b2")
            w3 = cp.tile([128, 4], bf16, tag="w3")
            w3dr = cp.tile([128, 2, 2, 16], f8, tag="w3dr")
            m168 = cp.tile([128, 168], f32, tag="m168")
            s168 = cp.tile([128, 168], f32, tag="s168")
            w1a = cp.tile([128, 2, 2, 512], f8, tag="w1a")
            w1b = cp.tile([128, 2, 2, 512], f8, tag="w1b")
            w2 = cp.tile([128, 2, 2, 512], f8, tag="w2")
            tg = cp.tile([128, 4, BR], f8, tag="tg")

            def deferred_loads(stage):
                if stage == 0:
                    nc.sync.dma_start(out=w1b[:],
                                      in_=d_w1b[:].transpose([1, 0, 2, 3]))
                    nc.sync.dma_start(out=id8[:], in_=d_id8[:])
                    nc.sync.dma_start(out=b1[:], in_=d_b1[:])
                    nc.sync.dma_start(out=b2[:], in_=d_b2[:])
                elif stage == 1:
                    nc.sync.dma_start(out=w2[:],
                                      in_=d_w2[:].transpose([1, 0, 2, 3]))
                    nc.sync.dma_start(out=w1a[:],
                                      in_=d_w1a[:].transpose([1, 0, 2, 3]))
                    nc.sync.dma_start(out=w3[:], in_=d_w3[:])
                    nc.sync.dma_start(out=w3dr[:],
                                      in_=d_w3dr[:].transpose([1, 0, 2, 3]))
                elif stage == 2:
                    nc.sync.dma_start(out=tg[:],
                                      in_=d_tg[:].transpose([1, 0, 2]))
                    nc.sync.dma_start(out=m168[:], in_=d_m168[:])
                    nc.sync.dma_start(out=s168[:], in_=d_s168[:])

            # parked L1-x pre-activations for the light blocks; their fp8
            # inputs stream through the same pool tag the full blocks use
            xb1 = [cp.tile([128, 4, 2, 512], f8, tag=f"xb1_{b}", name=f"xb1_{b}")
                   for b in range(LB)]
            xlb_cur = {}

            def light_xt(b):
                if b not in xlb_cur:
                    xt = mp.tile([128, 4, BR], f8, tag="negsx", bufs=2,
                                 name=f"negsx_lb{b}")
                    nc.sync.dma_start(
                        out=xt[:],
                        in_=d_negs[:, :, b * BR:(b + 1) * BR].transpose([1, 0, 2]))
                    xlb_cur[b] = xt
                return xlb_cur[b]

            predsT = cp.tile([128, 4, BR], f8, tag="preds")
            onesb = cp.tile([128, 4, R], bf16, tag="onesb")
            nc.gpsimd.memset(onesb[:], 1.0)

            # ------------------------------------------------ GRU
            # th-split: all th=0 matmuls of a step (which need only the
            # early half of h) are issued before any th=1 matmul, so the
            # previous step's elementwise tail overlaps real matmul work.
            do_gru = upto >= 1
            h_prev = h0
            for k in range(k_eff if do_gru else 0):
                gi = gp.tile([128, 12, R], bf16, tag="gi", bufs=2)
                nc.sync.dma_start(out=gi[:], in_=d_gi[k])

                rza = gp.tile([128, 8, R], bf16, tag="rza", bufs=1)
                rzps = [pg.tile([128, 4, R], f32, tag="pg", name=f"rzps{q}_{k}")
                        for q in range(2)]
                # th=0 wave (needs h half 0 only) + the gi identity-inject
                for q in range(2):
                    for jp in range(2):
                        nc.tensor.matmul(rzps[q][:, 2 * jp:2 * jp + 2, :], idbf[:],
                                         gi[:, 4 * q + 2 * jp:4 * q + 2 * jp + 2, :],
                                         start=True, stop=False)
                    for j in range(4):
                        gc = 4 * q + j
                        nc.tensor.matmul(
                            rzps[q][:, j, :], whh[:, 0, :, gc * 128:(gc + 1) * 128],
                            h_prev[:, 0:2, :],
                            start=False, stop=False, perf_mode=DR)
                # th=1 wave + sigmoid eviction per quad
                for q in range(2):
                    for j in range(4):
                        gc = 4 * q + j
                        nc.tensor.matmul(
                            rzps[q][:, j, :], whh[:, 1, :, gc * 128:(gc + 1) * 128],
                            h_prev[:, 2:4, :],
                            start=False, stop=True, perf_mode=DR)
                    nc.scalar.activation(rza[:, 4 * q:4 * q + 4, :], rzps[q][:],
                                         AF.Sigmoid)
                r_sb = rza[:, 0:4, :]
                z_sb = rza[:, 4:8, :]
                e_sb = gp.tile([128, 4, R], bf16, tag="e", bufs=1)
                nc.vector.tensor_mul(e_sb[:, 0:2, :], rza[:, 4:6, :],
                                     h_prev[:, 0:2, :])
                w1m = gp.tile([128, 4, R], bf16, tag="w1m", bufs=1)
                nc.vector.tensor_scalar(w1m[:], z_sb, -1.0, 1.0,
                                        op0=ALU.mult, op1=ALU.add)

                # n gates: one quad group, th-split as well
                nps = pg.tile([128, 4, R], f32, tag="pg", name=f"nps_{k}")
                for j in range(4):
                    nc.tensor.matmul(
                        nps[:, j, :], whh[:, 0, :, (8 + j) * 128:(9 + j) * 128],
                        h_prev[:, 0:2, :], start=True, stop=False, perf_mode=DR)
                for j in range(4):
                    nc.tensor.matmul(
                        nps[:, j, :], whh[:, 1, :, (8 + j) * 128:(9 + j) * 128],
                        h_prev[:, 2:4, :], start=False, stop=True, perf_mode=DR)

                t_sb = gp.tile([128, 4, R], bf16, tag="t", bufs=1)
                u_sb = gp.tile([128, 4, R], bf16, tag="u", bufs=1)
                g_sb = gp.tile([128, 4, R], bf16, tag="g", bufs=1)
                gw = gp.tile([128, 4, R], bf16, tag="gw", bufs=1)

                us = [fi for fi, u in enumerate(u_list) if u == k]
                if us:
                    fi0 = us[0]
                    h_new = predsT[:, :, fi0 * R:(fi0 + 1) * R]
                else:
                    h_new = gp.tile([128, 4, R], f8, tag="h8")

                # half 0 chain on Vector (it gates the next step's th=0
                # wave); half 1 has the th=0 wave's slack: GpSimd
                for pi in range(2):
                    for j in range(2):
                        c = 2 * pi + j
                        nc.vector.scalar_tensor_tensor(
                            t_sb[:, c, :], in0=nps[:, c, :], scalar=bgn[:, c:c + 1],
                            in1=r_sb[:, c, :], op0=ALU.add, op1=ALU.mult)
                        if pi == 0:
                            nc.vector.tensor_add(u_sb[:, c, :], gi[:, 8 + c, :],
                                                 t_sb[:, c, :])
                        else:
                            nc.gpsimd.tensor_add(u_sb[:, c, :], gi[:, 8 + c, :],
                                                 t_sb[:, c, :])
                    hh = slice(2 * pi, 2 * pi + 2)
                    nc.scalar.activation(g_sb[:, hh, :], u_sb[:, hh, :], AF.Tanh)
                    if pi == 0:
                        nc.vector.tensor_mul(gw[:, hh, :], g_sb[:, hh, :],
                                             w1m[:, hh, :])
                    else:
                        nc.gpsimd.tensor_mul(e_sb[:, 2:4, :], rza[:, 6:8, :],
                                             h_prev[:, 2:4, :])
                        nc.gpsimd.tensor_mul(gw[:, hh, :], g_sb[:, hh, :],
                                             w1m[:, hh, :])
                    nc.vector.tensor_add(h_new[:, hh, :], gw[:, hh, :],
                                         e_sb[:, hh, :])
                for fi in us[1:]:
                    nc.vector.tensor_copy(
                        predsT[:, :, fi * R:(fi + 1) * R], h_new[:])
                h_prev = h_new

                # interleaved light-block L1-x matmuls (fill the PE bubble)
                for (b, cc) in inter_sched.get(k, []):
                    xt = light_xt(b)
                    ps = pm.tile([128, 2, 512], f32, tag="pm")
                    for rt in range(2):
                        sl = slice(rt * 512, (rt + 1) * 512)
                        for th in range(2):
                            nc.tensor.matmul(
                                ps[:, rt, :],
                                w1b[:, th, :, cc * 128:(cc + 1) * 128],
                                xt[:, 2 * th:2 * th + 2, sl],
                                start=(th == 0), stop=(th == 1), perf_mode=DR)
                    if cc % 2 == 0:
                        nc.scalar.activation(xb1[b][:, cc, :, :], ps[:], AF.Copy)
                    else:
                        nc.vector.tensor_copy(xb1[b][:, cc, :, :], ps[:])
                deferred_loads(k)

            # ------------------------------------------- preds @ W1a + b1 cache
            cach = cp.tile([128, 4, 2, 512], bf16, tag="cach")
            if upto >= 2:
                for cc in range(4):
                    ps = pm.tile([128, 2, 512], f32, tag="pm")
                    for rt in range(2):
                        sl = slice(rt * 512, (rt + 1) * 512)
                        for th in range(2):
                            nc.tensor.matmul(
                                ps[:, rt, :],
                                w1a[:, th, :, cc * 128:(cc + 1) * 128],
                                predsT[:, 2 * th:2 * th + 2, sl],
                                start=(th == 0), stop=(th == 1), perf_mode=DR)
                    nc.scalar.activation(cach[:, cc, :, :], ps[:], AF.Identity,
                                         bias=b1[:, cc:cc + 1])

            # ------------------------------------------------ blocks
            flat = cp.tile([1, FL], bf16, tag="flat")
            tr168 = cp.tile([128, 168], bf16, tag="tr168")

            def l1_full(b, xt):
                y1 = mp.tile([128, 4, BR], f8, tag="y1", bufs=2)
                for cc in range(4):
                    ps = pm.tile([128, 2, 512], f32, tag="pm")
                    for rt in range(2):
                        sl = slice(rt * 512, (rt + 1) * 512)
                        nc.tensor.matmul(ps[:, rt, :], idbf[:],
                                         cach[:, cc, rt, :], start=True,
                                         stop=False)
                        for th in range(2):
                            nc.tensor.matmul(
                                ps[:, rt, :],
                                w1b[:, th, :, cc * 128:(cc + 1) * 128],
                                xt[:, 2 * th:2 * th + 2, sl],
                                start=False, stop=(th == 1), perf_mode=DR)
                    nc.vector.tensor_scalar(y1[:, cc, :], ps[:], 0.0, None,
                                            op0=ALU.max)
                return y1

            def l1_light(b):
                y1 = mp.tile([128, 4, BR], f8, tag="y1", bufs=2)
                for cc in range(4):
                    ps = pm.tile([128, 2, 512], f32, tag="pm")
                    for rt in range(2):
                        nc.tensor.matmul(ps[:, rt, :], id8[:],
                                         xb1[b][:, cc, rt, :], start=True,
                                         stop=False)
                        nc.tensor.matmul(ps[:, rt, :], idbf[:],
                                         cach[:, cc, rt, :], start=False,
                                         stop=True)
                    nc.vector.tensor_scalar(y1[:, cc, :], ps[:], 0.0, None,
                                            op0=ALU.max)
                return y1

            def l2_l3(b, y1):
                y2 = mp.tile([128, 4, BR], f8, tag="y2", bufs=2)
                for cc in range(4):
                    ps = pm.tile([128, 2, 512], f32, tag="pm")
                    for rt in range(2):
                        sl = slice(rt * 512, (rt + 1) * 512)
                        for th in range(2):
                            nc.tensor.matmul(
                                ps[:, rt, :],
                                w2[:, th, :, cc * 128:(cc + 1) * 128],
                                y1[:, 2 * th:2 * th + 2, sl],
                                start=(th == 0), stop=(th == 1), perf_mode=DR)
                    if cc < 2:
                        nc.vector.tensor_scalar(y2[:, cc, :], ps[:],
                                                b2[:, cc:cc + 1], 0.0,
                                                op0=ALU.add, op1=ALU.max)
                    else:
                        nc.scalar.activation(y2[:, cc, :], ps[:], AF.Relu,
                                             bias=b2[:, cc:cc + 1])
                # L3: w3-stationary DoubleRow, [1, 512] psum per half
                ps3 = pm.tile([128, 2, 512], f32, tag="pm", name=f"ps3_{b}")
                for j in range(2):
                    sl = slice(j * 512, (j + 1) * 512)
                    for t3 in range(2):
                        nc.tensor.matmul(
                            ps3[0:16, j, :], w3dr[:, t3, :, :],
                            y2[:, 2 * t3:2 * t3 + 2, sl],
                            start=(t3 == 0), stop=(t3 == 1), perf_mode=DR)
                nc.scalar.activation(flat[0:1, b * BR:(b + 1) * BR],
                                     ps3[0:1, :, :], AF.Copy)

            if upto >= 3:
                prev = None
                for b in range(NBLK):
                    if b < LB:
                        y1 = l1_light(b)
                    else:
                        if b < NEG:
                            xt = mp.tile([128, 4, BR], f8, tag="negsx", bufs=2)
                            for kc in range(4):
                                nc.sync.dma_start(
                                    out=xt[:, kc, :],
                                    in_=d_negs[kc][:, b * BR:(b + 1) * BR])
                        else:
                            xt = tg
                        y1 = l1_full(b, xt)
                    if prev is not None:
                        l2_l3(*prev)
                        # reshape-DMA half 0 once blocks 0..10 are flat
                        if prev[0] == 10:
                            for q in range(4):
                                nc.sync.dma_start(
                                    out=tr168[q * 32:(q + 1) * 32, 0:84],
                                    in_=flat[0:1, q * 2688:(q + 1) * 2688])
                    prev = (b, y1)
                l2_l3(*prev)
                for q in range(4):
                    nc.sync.dma_start(
                        out=tr168[q * 32:(q + 1) * 32, 84:168],
                        in_=flat[0:1, 10752 + q * 2688:10752 + (q + 1) * 2688])

            # --------------------------------------- softplus + masked sum
            # softplus(s*t) = relu(s*t) - ln(sigmoid(|t|)); one table switch
            out_sb = cp.tile([1, 4], f32, tag="out_sb")
            if upto >= 4:
                st = cp.tile([128, 168], f32, tag="st")
                nc.vector.scalar_tensor_tensor(
                    st[:], in0=tr168[:], scalar=b3f, in1=s168[:],
                    op0=ALU.add, op1=ALU.mult)
                ab = cp.tile([128, 168], f32, tag="ab")
                nc.scalar.activation(ab[:], st[:], AF.Abs)
                sg = cp.tile([128, 168], f32, tag="sg")
                nc.scalar.activation(sg[:], ab[:], AF.Sigmoid)
                ln = cp.tile([128, 168], f32, tag="ln")
                nc.scalar.activation(ln[:], sg[:], AF.Ln)
                rl = cp.tile([128, 168], f32, tag="rl")
                nc.vector.tensor_scalar(rl[:], st[:], 0.0, None, op0=ALU.max)
                sp = cp.tile([128, 168], f32, tag="sp")
                nc.vector.scalar_tensor_tensor(
                    sp[:], in0=ln[:], scalar=-1.0, in1=rl[:],
                    op0=ALU.mult, op1=ALU.add)
                wsp = cp.tile([128, 168], f32, tag="wsp")
                rsum = cp.tile([128, 1], f32, tag="rsum")
                nc.vector.scalar_tensor_tensor(
                    wsp[:], in0=sp[:], scalar=1.0, in1=m168[:],
                    op0=ALU.mult, op1=ALU.mult, accum_out=rsum[:])
                ones = cp.tile([128, 1], f32, tag="ones")
                nc.vector.memset(ones[:], 1.0)
                psf = pm.tile([128, 2, 512], f32, tag="pm", name="psf")
                nc.tensor.matmul(psf[0:1, 0, 0:1], rsum[:], ones[:],
                                 start=True, stop=True)
                nc.scalar.activation(out_sb[:], psf[0:1, 0, 0:4], AF.Copy)
            else:
                nc.vector.memset(out_sb[:], 0.0)
            nc.sync.dma_start(out=d_out[:], in_=out_sb[:])

    nc.finalize()
    return nc


def _get_program(u_list, k_eff, b3f):
    key = (tuple(u_list), k_eff, float(b3f))
    if key not in _PROGRAM_CACHE:
        _PROGRAM_CACHE[key] = _build_program(u_list, k_eff, b3f)
    return _PROGRAM_CACHE[key]


# ------------------------------------------------------------------ kernel

def kernel(**inputs):
    u_list = [int(x) for x in np.asarray(inputs["unroll_subsample"]).reshape(-1)]
    k_eff = max(u_list) + 1
    w = _prep_weights(inputs)
    nc = _get_program(u_list, k_eff, w["b3f"])

    wmaps = {k: v for k, v in w.items() if k != "b3f"}
    in_maps = []
    D = 0.0
    for c in range(NC):
        m = dict(wmaps)
        cm, dc = _prep_core(c, inputs, u_list, k_eff)
        m.update(cm)
        in_maps.append(m)
        D += dc

    res = bass_utils.run_bass_kernel_spmd(nc, in_maps, list(range(NC)))
    WS = 0.0
    for c in range(NC):
        o = np.asarray(res.results[c]["out"], np.float64)
        WS += o[0, 0]
    loss = COEFF * WS / D
    return np.float32(loss)


# revision 27
# speedup vs baseline: 1.0197x; 1.0046x over previous
"""Trainium2 Bass kernel for the CPCA auxiliary loss (nn_CPCA_51754355917033).

Strategy (data-parallel over the env/batch dim n, 16 envs per core):
  - Host side: every gather is baked into per-core contiguous device
    inputs (embedding lookup folded into GIE = emb @ W_ih.T + b_ih with
    the r/z parts of b_hh also folded in, h0 gather, target gather, the
    negative gather, the forward mask and the loss denominator).
  - Device side GRU (gate dim on partitions): gi is injected into the
    gate PSUM with an identity matmul so the r/z eviction is a single
    Sigmoid straight out of PSUM; h carried in fp8; elementwise tail
    split across Vector/Scalar/GpSimd; step-u states written directly
    into the preds buffer.
  - MLP blocks: preds @ W1a + b1 computed once, re-injected per block
    into the L1 PSUM with identity matmuls, so evictions are single
    relu-cast ops.  L3 is a w3-stationary matmul giving [1, 1024]
    logits per block, copied into a flat row; at the end one reshape
    DMA spreads the row over 128 partitions for a whole-tensor
    softplus + masked-sum (one act-table switch total).
  - Host combines the 8 cores' weighted sums with the host-side
    denominator into the final scalar.
"""

import numpy as np
import ml_dtypes

import concourse.bass as bass
import concourse.mybir as mybir
import concourse.tile as tile
from concourse import bacc
from concourse import bass_utils

BF16 = ml_dtypes.bfloat16
F8 = ml_dtypes.float8_e4m3
DT = mybir.dt
AF = mybir.ActivationFunctionType
ALU = mybir.AluOpType

N, T, H, K, S, F, EMB, NLOG, NEG = 128, 512, 512, 16, 16, 4, 32, 18, 20
COEFF = 0.1
NC = 8
NPC = N // NC          # envs per core
R = NPC * S            # GRU rows per core (256)
L = T - 1
NBLK = NEG + 1         # 20 negative g-blocks + 1 positive block
BR = F * R             # rows per block (1024)
FL = NBLK * BR         # flat logits row (21504)
LB = 10                # light blocks: L1-x matmuls interleaved into the GRU

_PROGRAM_CACHE = {}


# ----------------------------------------------------------------- host prep

def _prep_core(c, inputs, u_list, k_eff):
    acts = np.asarray(inputs["actions"])[..., 0]
    nd = np.asarray(inputs["not_dones"])[..., 0]
    ri = np.asarray(inputs["rnn_inputs"], np.float32)
    ro = np.asarray(inputs["rnn_outputs"], np.float32)
    ti = np.asarray(inputs["time_subsample"]).astype(np.int64)
    neg_idx = np.asarray(inputs["neg_idx"]).astype(np.int64)
    emb_tab = np.asarray(inputs["action_embed"], np.float32)

    ns = slice(c * NPC, (c + 1) * NPC)
    idx = np.arange(k_eff)[:, None] + ti[None, :]          # (k_eff, S)

    # gi = emb @ W_ih.T folded on host: a gather from the 18-row table.
    # GIE = action_embed @ W_ih.T + b_ih (+ b_hh for the r/z gates,
    # legal because sigmoid(ir + hr + b_ih + b_hh) is a plain sum there).
    W_ih = np.asarray(inputs["W_ih"], np.float32)
    b_ih = np.asarray(inputs["b_ih"], np.float32)
    b_hh = np.asarray(inputs["b_hh"], np.float32)
    GIE = np.zeros((NLOG + 1, 1536), np.float32)
    GIE[:NLOG] = emb_tab @ W_ih.T + b_ih
    GIE[NLOG] = b_ih
    GIE[:, :1024] += b_hh[None, :1024]
    act_ext = np.full((NPC, L + K), NLOG, np.int64)
    act_ext[:, :L] = acts[ns, :L]
    AI = act_ext[:, idx]                                   # (NPC, k_eff, S)
    gi_all = GIE[AI.transpose(1, 0, 2).reshape(k_eff, R)]  # (k_eff, R, 1536)
    giT = np.ascontiguousarray(
        gi_all.transpose(0, 2, 1).reshape(k_eff, 12, 128, R)
        .transpose(0, 2, 1, 3)).astype(BF16)               # (k_eff,128,12,R)

    H0 = ro[ns][:, ti]                                     # (NPC, S, H)
    h0T = np.ascontiguousarray(
        H0.transpose(2, 0, 1).reshape(4, 128, R)).astype(F8)

    ri_ext = np.zeros((NPC, L + K, H), np.float32)
    ri_ext[:, :L] = ri[ns, 1:]
    idx2 = np.asarray(u_list)[:, None] + ti[None, :]       # (F, S)
    TG = ri_ext[:, idx2]                                   # (NPC, F, S, H)
    tgT = np.ascontiguousarray(
        TG.transpose(3, 1, 0, 2).reshape(H, BR).reshape(4, 128, BR)).astype(F8)

    ni = neg_idx.reshape(F, N, S, NEG)[:, ns]              # (F, NPC, S, NEG)
    P = ni.transpose(3, 0, 1, 2).reshape(-1)               # cols in (g, f, j) order
    negs = ri.reshape(N * T, H)[P]
    negsT = np.ascontiguousarray(negs.T.reshape(4, 128, NEG * BR)).astype(F8)

    # forward mask + denominator on host (mask depends only on inputs)
    nd_ext = np.zeros((NPC, L + K), np.float32)
    nd_ext[:, :L] = nd[ns, :L]
    G = nd_ext[:, idx]                                     # (NPC, k_eff, S)
    ndv = G.transpose(1, 0, 2).reshape(k_eff, R) > 0       # (k_eff, R)
    cum = np.cumprod(ndv.astype(np.float32), axis=0)       # (k_eff, R)
    fm = cum[np.asarray(u_list)]                           # (F, R)
    fm_flat = fm.reshape(BR).astype(np.float32)            # f-major rows
    denom = float(fm_flat.sum())

    # weighted mask + sign in the [128, 168] transposed-flat layout:
    # tr[p, h*84 + cc] = flat[h*10752 + p*84 + cc]
    q = np.arange(FL)
    wq = fm_flat[q % BR] * np.where(q // BR < NEG, 1.0 / NEG, 1.0)
    sq = np.where(q // BR < NEG, 1.0, -1.0)
    mask168 = np.ascontiguousarray(
        wq.reshape(2, 128, 84).transpose(1, 0, 2).reshape(128, 168)
    ).astype(np.float32)
    sgn168 = np.ascontiguousarray(
        sq.reshape(2, 128, 84).transpose(1, 0, 2).reshape(128, 168)
    ).astype(np.float32)

    return dict(giT=giT, h0T8=h0T, tgT=tgT, negsT=negsT,
                mask168=mask168, sgn168=sgn168), denom


def _prep_weights(inputs):
    W_hh = np.asarray(inputs["W_hh"], np.float32)
    b_hh = np.asarray(inputs["b_hh"], np.float32)
    W1 = np.asarray(inputs["W1"], np.float32)
    b1 = np.asarray(inputs["b1"], np.float32)
    W2 = np.asarray(inputs["W2"], np.float32)
    b2 = np.asarray(inputs["b2"], np.float32)
    W3 = np.asarray(inputs["W3"], np.float32)
    b3 = np.asarray(inputs["b3"], np.float32)

    d = {}
    d["w_hh8"] = np.ascontiguousarray(
        W_hh.T.reshape(2, 2, 128, 1536).transpose(0, 2, 1, 3)).astype(F8)
    def pack8(WT):
        # [t, ki, ko, m] with contract index = t*256 + ko*128 + ki
        return np.ascontiguousarray(
            WT.reshape(2, 2, 128, WT.shape[1]).transpose(0, 2, 1, 3)).astype(F8)
    d["w1a8"] = pack8(W1[:, :512].T.copy())
    d["w1b8"] = pack8(W1[:, 512:].T.copy())
    d["w28"] = pack8(W2.T.copy())
    d["w3T"] = np.ascontiguousarray(W3[0].reshape(4, 128).T).astype(BF16)
    # DoubleRow-packed w3: [t, ki, ko, 16] (cols 1..15 zero-padded so the
    # dual-fp8 ldweights stride restrictions hold); contract = t*256+ko*128+ki
    w3dr = np.zeros((2, 128, 2, 16), np.float32)
    w3dr[:, :, :, 0] = W3[0].reshape(2, 2, 128).transpose(0, 2, 1)
    d["w3dr"] = w3dr.astype(F8)
    # n-gate hidden biases only (r/z parts folded into GIE on host)
    bgn = np.zeros((128, 4), np.float32)
    for cc in range(4):
        bgn[:, cc] = b_hh[1024 + cc * 128:1024 + (cc + 1) * 128]
    d["bgn"] = bgn
    d["b1T"] = np.ascontiguousarray(b1.reshape(4, 128).T).astype(np.float32)
    d["b2T"] = np.ascontiguousarray(b2.reshape(4, 128).T).astype(np.float32)
    d["idbf"] = np.eye(128, dtype=BF16)
    d["id8"] = np.eye(128, dtype=F8)
    d["b3f"] = float(b3.reshape(-1)[0])
    return d


# ------------------------------------------------------------- device program

def _build_program(u_list, k_eff, b3f, upto=4):
    nc = bacc.Bacc("TRN2", target_bir_lowering=False, debug=False, num_devices=NC)

    di = {}
    def inp(name, shape, dt):
        di[name] = nc.dram_tensor(name, list(shape), dt, kind="ExternalInput")
        return di[name]

    d_whh = inp("w_hh8", (2, 128, 2, 1536), DT.float8e4)
    d_w1a = inp("w1a8", (2, 128, 2, 512), DT.float8e4)
    d_w1b = inp("w1b8", (2, 128, 2, 512), DT.float8e4)
    d_w2 = inp("w28", (2, 128, 2, 512), DT.float8e4)
    d_w3 = inp("w3T", (128, 4), DT.bfloat16)
    d_w3dr = inp("w3dr", (2, 128, 2, 16), DT.float8e4)
    d_bgn = inp("bgn", (128, 4), DT.float32)
    d_b1 = inp("b1T", (128, 4), DT.float32)
    d_b2 = inp("b2T", (128, 4), DT.float32)
    d_idbf = inp("idbf", (128, 128), DT.bfloat16)
    d_id8 = inp("id8", (128, 128), DT.float8e4)
    d_gi = inp("giT", (k_eff, 128, 12, R), DT.bfloat16)
    d_h0 = inp("h0T8", (4, 128, R), DT.float8e4)
    d_tg = inp("tgT", (4, 128, BR), DT.float8e4)
    d_negs = inp("negsT", (4, 128, NEG * BR), DT.float8e4)
    d_m168 = inp("mask168", (128, 168), DT.float32)
    d_s168 = inp("sgn168", (128, 168), DT.float32)
    d_out = nc.dram_tensor("out", [1, 4], DT.float32, kind="ExternalOutput")

    f32 = DT.float32
    bf16 = DT.bfloat16
    f8 = DT.float8e4
    DR = mybir.MatmulPerfMode.DoubleRow

    # which GRU step each light block's (b, cc) L1-x group is issued after
    inter_sched = {}
    if k_eff >= 2:
        pairs = [(b, cc) for b in range(LB) for cc in range(4)]
        steps = list(range(1, k_eff))
        for i, (b, cc) in enumerate(pairs):
            st = steps[min(int(i * len(steps) / len(pairs)), len(steps) - 1)]
            inter_sched.setdefault(st, []).append((b, cc))

    with tile.TileContext(nc) as tc:
        with (
            tc.tile_pool(name="const", bufs=1) as cp,
            tc.tile_pool(name="gruw", bufs=2) as gp,
            tc.tile_pool(name="mlpw", bufs=2) as mp,
            tc.tile_pool(name="psg", bufs=2, space="PSUM") as pg,
            tc.tile_pool(name="psm", bufs=2, space="PSUM") as pm,
        ):
            # ------------------------------------------------ constant loads
            # single transposed-AP DMAs; only GRU-critical ones up front,
            # the rest deferred into the first two GRU steps (the Sync
            # engine issues dma_starts serially at ~0.65us each)
            idbf = cp.tile([128, 128], bf16, tag="idbf")
            nc.sync.dma_start(out=idbf[:], in_=d_idbf[:])
            h0 = cp.tile([128, 4, R], f8, tag="h0")
            nc.sync.dma_start(out=h0[:], in_=d_h0[:].transpose([1, 0, 2]))
            whh = cp.tile([128, 2, 2, 1536], f8, tag="whh")
            bgn = cp.tile([128, 4], f32, tag="bgn")
            nc.sync.dma_start(out=bgn[:], in_=d_bgn[:])
            id8 = cp.tile([128, 128], f8, tag="id8")
            b1 = cp.tile([128, 4], f32, tag="b1")
            b2 = cp.tile([128, 4], f32, tag="b2")
            w3 = cp.tile([128, 4], bf16, tag="w3")
            w3dr = cp.tile([128, 2, 2, 16], f8, tag="w3dr")
            m168 = cp.tile([128, 168], f32, tag="m168")
            s168 = cp.tile([128, 168], f32, tag="s168")
            w1a = cp.tile([128, 2, 2, 512], f8, tag="w1a")
            w1b = cp.tile([128, 2, 2, 512], f8, tag="w1b")
            w2 = cp.tile([128, 2, 2, 512], f8, tag="w2")
            tg = cp.tile([128, 4, BR], f8, tag="tg")

            def deferred_loads(stage):
                if stage == 0:
                    nc.sync.dma_start(out=w1b[:],
                                      in_=d_w1b[:].transpose([1, 0, 2, 3]))
                    nc.sync.dma_start(out=id8[:], in_=d_id8[:])
                    nc.sync.dma_start(out=b1[:], in_=d_b1[:])
                    nc.sync.dma_start(out=b2[:], in_=d_b2[:])
                elif stage == 1:
                    nc.sync.dma_start(out=w2[:],
                                      in_=d_w2[:].transpose([1, 0, 2, 3]))
                    nc.sync.dma_start(out=w1a[:],
                                      in_=d_w1a[:].transpose([1, 0, 2, 3]))
                    nc.sync.dma_start(out=w3[:], in_=d_w3[:])
                    nc.sync.dma_start(out=w3dr[:],
                                      in_=d_w3dr[:].transpose([1, 0, 2, 3]))
                elif stage == 2:
                    nc.sync.dma_start(out=tg[:],
                                      in_=d_tg[:].transpose([1, 0, 2]))
                    nc.sync.dma_start(out=m168[:], in_=d_m168[:])
                    nc.sync.dma_start(out=s168[:], in_=d_s168[:])

            # parked L1-x pre-activations for the light blocks; their fp8
            # inputs stream through the same pool tag the full blocks use
            xb1 = [cp.tile([128, 4, 2, 512], f8, tag=f"xb1_{b}", name=f"xb1_{b}")
                   for b in range(LB)]
            xlb_cur = {}

            def light_xt(b):
                if b not in xlb_cur:
                    xt = mp.tile([128, 4, BR], f8, tag="negsx", bufs=2,
                                 name=f"negsx_lb{b}")
                    nc.sync.dma_start(
                        out=xt[:],
                        in_=d_negs[:, :, b * BR:(b + 1) * BR].transpose([1, 0, 2]))
                    xlb_cur[b] = xt
                return xlb_cur[b]

            predsT = cp.tile([128, 4, BR], f8, tag="preds")
            onesb = cp.tile([128, 4, R], bf16, tag="onesb")
            nc.gpsimd.memset(onesb[:], 1.0)

            # ------------------------------------------------ GRU
            # th-split: all th=0 matmuls of a step (which need only the
            # early half of h) are issued before any th=1 matmul, so the
            # previous step's elementwise tail overlaps real matmul work.
            do_gru = upto >= 1
            h_prev = h0
            for k in range(k_eff if do_gru else 0):
                gi = gp.tile([128, 12, R], bf16, tag="gi", bufs=2)
                nc.sync.dma_start(out=gi[:], in_=d_gi[k])
                if k == 0:
                    nc.sync.dma_start(out=whh[:, 0, :, :], in_=d_whh[0])
                    nc.sync.dma_start(out=whh[:, 1, :, :], in_=d_whh[1])

                rza = gp.tile([128, 8, R], bf16, tag="rza", bufs=1)
                rzps = [pg.tile([128, 4, R], f32, tag="pg", name=f"rzps{q}_{k}")
                        for q in range(2)]
                # th=0 wave (needs h half 0 only) + the gi identity-inject
                for q in range(2):
                    for jp in range(2):
                        nc.tensor.matmul(rzps[q][:, 2 * jp:2 * jp + 2, :], idbf[:],
                                         gi[:, 4 * q + 2 * jp:4 * q + 2 * jp + 2, :],
                                         start=True, stop=False)
                    for j in range(4):
                        gc = 4 * q + j
                        nc.tensor.matmul(
                            rzps[q][:, j, :], whh[:, 0, :, gc * 128:(gc + 1) * 128],
                            h_prev[:, 0:2, :],
                            start=False, stop=False, perf_mode=DR)
                # th=1 wave + sigmoid eviction per quad
                for q in range(2):
                    for j in range(4):
                        gc = 4 * q + j
                        nc.tensor.matmul(
                            rzps[q][:, j, :], whh[:, 1, :, gc * 128:(gc + 1) * 128],
                            h_prev[:, 2:4, :],
                            start=False, stop=True, perf_mode=DR)
                    nc.scalar.activation(rza[:, 4 * q:4 * q + 4, :], rzps[q][:],
                                         AF.Sigmoid)
                r_sb = rza[:, 0:4, :]
                z_sb = rza[:, 4:8, :]
                e_sb = gp.tile([128, 4, R], bf16, tag="e", bufs=1)
                nc.vector.tensor_mul(e_sb[:, 0:2, :], rza[:, 4:6, :],
                                     h_prev[:, 0:2, :])
                w1m = gp.tile([128, 4, R], bf16, tag="w1m", bufs=1)
                nc.vector.tensor_scalar(w1m[:], z_sb, -1.0, 1.0,
                                        op0=ALU.mult, op1=ALU.add)

                # n gates: one quad group, th-split as well
                nps = pg.tile([128, 4, R], f32, tag="pg", name=f"nps_{k}")
                for j in range(4):
                    nc.tensor.matmul(
                        nps[:, j, :], whh[:, 0, :, (8 + j) * 128:(9 + j) * 128],
                        h_prev[:, 0:2, :], start=True, stop=False, perf_mode=DR)
                for j in range(4):
                    nc.tensor.matmul(
                        nps[:, j, :], whh[:, 1, :, (8 + j) * 128:(9 + j) * 128],
                        h_prev[:, 2:4, :], start=False, stop=True, perf_mode=DR)

                t_sb = gp.tile([128, 4, R], bf16, tag="t", bufs=1)
                u_sb = gp.tile([128, 4, R], bf16, tag="u", bufs=1)
                g_sb = gp.tile([128, 4, R], bf16, tag="g", bufs=1)
                gw = gp.tile([128, 4, R], bf16, tag="gw", bufs=1)

                us = [fi for fi, u in enumerate(u_list) if u == k]
                if us:
                    fi0 = us[0]
                    h_new = predsT[:, :, fi0 * R:(fi0 + 1) * R]
                else:
                    h_new = gp.tile([128, 4, R], f8, tag="h8")

                # half 0 chain on Vector (it gates the next step's th=0
                # wave); half 1 has the th=0 wave's slack: GpSimd
                for pi in range(2):
                    for j in range(2):
                        c = 2 * pi + j
                        nc.vector.scalar_tensor_tensor(
                            t_sb[:, c, :], in0=nps[:, c, :], scalar=bgn[:, c:c + 1],
                            in1=r_sb[:, c, :], op0=ALU.add, op1=ALU.mult)
                        if pi == 0:
                            nc.vector.tensor_add(u_sb[:, c, :], gi[:, 8 + c, :],
                                                 t_sb[:, c, :])
                        else:
                            nc.gpsimd.tensor_add(u_sb[:, c, :], gi[:, 8 + c, :],
                                                 t_sb[:, c, :])
                    hh = slice(2 * pi, 2 * pi + 2)
                    nc.scalar.activation(g_sb[:, hh, :], u_sb[:, hh, :], AF.Tanh)
                    if pi == 0:
                        nc.vector.tensor_mul(gw[:, hh, :], g_sb[:, hh, :],
                                             w1m[:, hh, :])
                    else:
                        nc.gpsimd.tensor_mul(e_sb[:, 2:4, :], rza[:, 6:8, :],
                                             h_prev[:, 2:4, :])
                        nc.gpsimd.tensor_mul(gw[:, hh, :], g_sb[:, hh, :],
                                             w1m[:, hh, :])
                    nc.vector.tensor_add(h_new[:, hh, :], gw[:, hh, :],
                                         e_sb[:, hh, :])
                for fi in us[1:]:
                    nc.vector.tensor_copy(
                        predsT[:, :, fi * R:(fi + 1) * R], h_new[:])
                h_prev = h_new

                # interleaved light-block L1-x matmuls (fill the PE bubble)
                for (b, cc) in inter_sched.get(k, []):
                    xt = light_xt(b)
                    ps = pm.tile([128, 2, 512], f32, tag="pm")
                    for rt in range(2):
                        sl = slice(rt * 512, (rt + 1) * 512)
                        for th in range(2):
                            nc.tensor.matmul(
                                ps[:, rt, :],
                                w1b[:, th, :, cc * 128:(cc + 1) * 128],
                                xt[:, 2 * th:2 * th + 2, sl],
                                start=(th == 0), stop=(th == 1), perf_mode=DR)
                    if cc % 2 == 0:
                        nc.scalar.activation(xb1[b][:, cc, :, :], ps[:], AF.Copy)
                    else:
                        nc.vector.tensor_copy(xb1[b][:, cc, :, :], ps[:])
                deferred_loads(k)

            # ------------------------------------------- preds @ W1a + b1 cache
            cach = cp.tile([128, 4, 2, 512], bf16, tag="cach")
            if upto >= 2:
                for cc in range(4):
                    ps = pm.tile([128, 2, 512], f32, tag="pm")
                    for rt in range(2):
                        sl = slice(rt * 512, (rt + 1) * 512)
                        for th in range(2):
                            nc.tensor.matmul(
                                ps[:, rt, :],
                                w1a[:, th, :, cc * 128:(cc + 1) * 128],
                                predsT[:, 2 * th:2 * th + 2, sl],
                                start=(th == 0), stop=(th == 1), perf_mode=DR)
                    nc.scalar.activation(cach[:, cc, :, :], ps[:], AF.Identity,
                                         bias=b1[:, cc:cc + 1])

            # ------------------------------------------------ blocks
            flat = cp.tile([1, FL], bf16, tag="flat")
            tr168 = cp.tile([128, 168], bf16, tag="tr168")
            st = cp.tile([128, 168], f32, tag="st")
            ab = cp.tile([128, 168], f32, tag="ab")
            ex = cp.tile([128, 168], f32, tag="ex")
            lg = cp.tile([128, 168], f32, tag="lg")
            rl = cp.tile([128, 168], f32, tag="rl")
            sp = cp.tile([128, 168], f32, tag="sp")
            wsp = cp.tile([128, 168], f32, tag="wsp")
            rsum = cp.tile([128, 2], f32, tag="rsum")

            def softplus_half(h):
                # softplus(s*t) = relu(s*t) + ln(1 + exp(-|t|)) — Abs, Exp,
                # Ln, Relu, Copy all live in one act table: single switch
                c = slice(h * 84, h * 84 + 84)
                nc.vector.scalar_tensor_tensor(
                    st[:, c], in0=tr168[:, c], scalar=b3f, in1=s168[:, c],
                    op0=ALU.add, op1=ALU.mult)
                nc.scalar.activation(ab[:, c], st[:, c], AF.Abs)
                nc.scalar.activation(ex[:, c], ab[:, c], AF.Exp, scale=-1.0)
                nc.scalar.activation(lg[:, c], ex[:, c], AF.Ln, bias=1.0)
                nc.vector.tensor_scalar(rl[:, c], st[:, c], 0.0, None,
                                        op0=ALU.max)
                nc.vector.tensor_add(sp[:, c], rl[:, c], lg[:, c])
                nc.vector.scalar_tensor_tensor(
                    wsp[:, c], in0=sp[:, c], scalar=1.0, in1=m168[:, c],
                    op0=ALU.mult, op1=ALU.mult, accum_out=rsum[:, h:h + 1])

            def l1_full(b, xt):
                y1 = mp.tile([128, 4, BR], f8, tag="y1", bufs=2)
                for cc in range(4):
                    ps = pm.tile([128, 2, 512], f32, tag="pm")
                    for rt in range(2):
                        sl = slice(rt * 512, (rt + 1) * 512)
                        nc.tensor.matmul(ps[:, rt, :], idbf[:],
                                         cach[:, cc, rt, :], start=True,
                                         stop=False)
                        for th in range(2):
                            nc.tensor.matmul(
                                ps[:, rt, :],
                                w1b[:, th, :, cc * 128:(cc + 1) * 128],
                                xt[:, 2 * th:2 * th + 2, sl],
                                start=False, stop=(th == 1), perf_mode=DR)
                    nc.vector.tensor_scalar(y1[:, cc, :], ps[:], 0.0, None,
                                            op0=ALU.max)
                return y1

            def l1_light(b):
                y1 = mp.tile([128, 4, BR], f8, tag="y1", bufs=2)
                for cc in range(4):
                    ps = pm.tile([128, 2, 512], f32, tag="pm")
                    for rt in range(2):
                        nc.tensor.matmul(ps[:, rt, :], id8[:],
                                         xb1[b][:, cc, rt, :], start=True,
                                         stop=False)
                        nc.tensor.matmul(ps[:, rt, :], idbf[:],
                                         cach[:, cc, rt, :], start=False,
                                         stop=True)
                    nc.vector.tensor_scalar(y1[:, cc, :], ps[:], 0.0, None,
                                            op0=ALU.max)
                return y1

            def l2_l3(b, y1):
                y2 = mp.tile([128, 4, BR], f8, tag="y2", bufs=2)
                for cc in range(4):
                    ps = pm.tile([128, 2, 512], f32, tag="pm")
                    for rt in range(2):
                        sl = slice(rt * 512, (rt + 1) * 512)
                        for th in range(2):
                            nc.tensor.matmul(
                                ps[:, rt, :],
                                w2[:, th, :, cc * 128:(cc + 1) * 128],
                                y1[:, 2 * th:2 * th + 2, sl],
                                start=(th == 0), stop=(th == 1), perf_mode=DR)
                    if cc < 2:
                        nc.vector.tensor_scalar(y2[:, cc, :], ps[:],
                                                b2[:, cc:cc + 1], 0.0,
                                                op0=ALU.add, op1=ALU.max)
                    else:
                        nc.scalar.activation(y2[:, cc, :], ps[:], AF.Relu,
                                             bias=b2[:, cc:cc + 1])
                # L3: w3-stationary DoubleRow, [1, 512] psum per half
                ps3 = pm.tile([128, 2, 512], f32, tag="pm", name=f"ps3_{b}")
                for j in range(2):
                    sl = slice(j * 512, (j + 1) * 512)
                    for t3 in range(2):
                        nc.tensor.matmul(
                            ps3[0:16, j, :], w3dr[:, t3, :, :],
                            y2[:, 2 * t3:2 * t3 + 2, sl],
                            start=(t3 == 0), stop=(t3 == 1), perf_mode=DR)
                nc.scalar.activation(flat[0:1, b * BR:(b + 1) * BR],
                                     ps3[0:1, :, :], AF.Copy)

            if upto >= 3:
                prev = None
                for b in range(NBLK):
                    if b < LB:
                        y1 = l1_light(b)
                    else:
                        if b < NEG:
                            xt = mp.tile([128, 4, BR], f8, tag="negsx", bufs=2)
                            for kc in range(4):
                                nc.sync.dma_start(
                                    out=xt[:, kc, :],
                                    in_=d_negs[kc][:, b * BR:(b + 1) * BR])
                        else:
                            xt = tg
                        y1 = l1_full(b, xt)
                    if prev is not None:
                        l2_l3(*prev)
                        # reshape-DMA half 0 once blocks 0..10 are flat,
                        # then its softplus runs under blocks 11..20
                        if prev[0] == 10:
                            for q in range(4):
                                nc.sync.dma_start(
                                    out=tr168[q * 32:(q + 1) * 32, 0:84],
                                    in_=flat[0:1, q * 2688:(q + 1) * 2688])
                            softplus_half(0)
                    prev = (b, y1)
                l2_l3(*prev)
                for q in range(4):
                    nc.sync.dma_start(
                        out=tr168[q * 32:(q + 1) * 32, 84:168],
                        in_=flat[0:1, 10752 + q * 2688:10752 + (q + 1) * 2688])
                softplus_half(1)

            # --------------------------------------- final combine
            # (softplus halves were computed inline; see softplus_half)
            out_sb = cp.tile([1, 4], f32, tag="out_sb")
            if upto >= 4:
                ones = cp.tile([128, 1], f32, tag="ones")
                nc.vector.memset(ones[:], 1.0)
                psf = pm.tile([128, 2, 512], f32, tag="pm", name="psf")
                nc.tensor.matmul(psf[0:1, 0, 0:2], ones[:], rsum[:],
                                 start=True, stop=True)
                nc.scalar.activation(out_sb[:], psf[0:1, 0, 0:4], AF.Copy)
            else:
                nc.vector.memset(out_sb[:], 0.0)
            nc.sync.dma_start(out=d_out[:], in_=out_sb[:])

    nc.finalize()
    return nc


def _get_program(u_list, k_eff, b3f):
    key = (tuple(u_list), k_eff, float(b3f))
    if key not in _PROGRAM_CACHE:
        _PROGRAM_CACHE[key] = _build_program(u_list, k_eff, b3f)
    return _PROGRAM_CACHE[key]


# ------------------------------------------------------------------ kernel

def kernel(**inputs):
    u_list = [int(x) for x in np.asarray(inputs["unroll_subsample"]).reshape(-1)]
    k_eff = max(u_list) + 1
    w = _prep_weights(inputs)
    nc = _get_program(u_list, k_eff, w["b3f"])

    wmaps = {k: v for k, v in w.items() if k != "b3f"}
    in_maps = []
    D = 0.0
    for c in range(NC):
        m = dict(wmaps)
        cm, dc = _prep_core(c, inputs, u_list, k_eff)
        m.update(cm)
        in_maps.append(m)
        D += dc

    res = bass_utils.run_bass_kernel_spmd(nc, in_maps, list(range(NC)))
    WS = 0.0
    for c in range(NC):
        o = np.asarray(res.results[c]["out"], np.float64)
        WS += o[0, 0] + o[0, 1]
    loss = COEFF * WS / D
    return np.float32(loss)
